# revision 19
# baseline (speedup 1.0000x reference)
"""NTM-style scatter_memory kernel for Trainium2, 8 NeuronCores.

Reference math (per problem nn_Memory_72249939853591):
    k = tanh(k); g = sigmoid(g); s = softmax(s); gamma = relu(gamma) + 1
    dots = memory @ k
    wc = softmax(dots / (max(|row|,eps) * max(|k|,eps)))
    wg = g * wc + (1 - g) * prev_w
    ws = s0 * roll(wg, 1) + s1 * wg + s2 * roll(wg, -1)
    w = ws ** gamma; w /= sum(w)
    new_memory = memory * (1 - w e^T) + w a^T
    returns (new_memory, w)

Sharding: memory / prev_w / w row-sharded over 8 cores (16384 rows each).
Cross-core communication: two tiny AllGathers (softmax denominator +
1-element roll halos, then the sharpening normalizer).

Per-core layouts:
  TM (tile-major):  (128, NT) tile t[p, i] = vec[i*128 + p]   -- matches
      how (128,256) memory sub-tiles land in SBUF partitions.
  PM (part-major):  (NT, 128) tile t[q, s] = vec[q*128 + s]   -- the roll
      is a free-axis shift here; carries cross partitions via tiny PE
      shift-matrix matmuls. TM <-> PM is a PE transpose.
"""

import numpy as np

import concourse.bacc as bacc
import concourse.bass as bass
import concourse.mybir as mybir
import concourse.tile as tile
from concourse import bass_utils, masks
from concourse.bass import _add_dep_helper


def _chain(prev, cur):
    """Order-only dep: keep same-engine streams in emission order so the
    tile scheduler can't round-robin them against DMA arrival order."""
    if prev is not None:
        _add_dep_helper(cur.ins, prev.ins, sync=False,
                        reason="keep engine stream in emission order")
    return cur

f32 = mybir.dt.float32
N_CORES = 8
M = 256
EPS = 1e-8
# engine balance knobs (measured costs: ACT sq+acc 814ns, DVE stt+acc 528ns,
# DVE stt 446ns, GP tt 766ns, ACT P1 535ns per 128x256 f32 tile)
SQ_DVE_EVERY = 5   # every 5th sub-tile's row-sumsq on DVE instead of ACT
MUL_DVE_EVERY = 12  # every 4th sub-tile's erase-mul on DVE instead of GPSIMD


def build_nc(R, stage=4):
    """Build + compile the per-core Bass module for R rows per core.

    stage < 4 truncates the pipeline for hardware hang bisection:
      1 = consts + phase A only (copy-through outputs)
      2 = + phase B, collective 1, selector matmuls, halo scalars
      3 = + transpose/wg/roll/sharpen, collective 2
      4 = full kernel
    """
    NT = R // 128          # number of 128-row sub-tiles
    JB = 8                 # sub-tiles per big DMA tile
    NB = NT // JB          # big tiles (1 MiB each)
    assert NB * JB == NT
    AluOp = mybir.AluOpType
    Act = mybir.ActivationFunctionType

    nc = bacc.Bacc("TRN2", target_bir_lowering=False, debug=False,
                   num_devices=N_CORES)

    mem_in = nc.dram_tensor("mem_in", [R, M], f32, kind="ExternalInput")
    pw_in = nc.dram_tensor("pw_in", [R], f32, kind="ExternalInput")
    k_in = nc.dram_tensor("k_in", [M], f32, kind="ExternalInput")
    e_in = nc.dram_tensor("e_in", [M], f32, kind="ExternalInput")
    a_in = nc.dram_tensor("a_in", [M], f32, kind="ExternalInput")
    g_in = nc.dram_tensor("g_in", [1], f32, kind="ExternalInput")
    s_in = nc.dram_tensor("s_in", [3], f32, kind="ExternalInput")
    gam_in = nc.dram_tensor("gam_in", [1], f32, kind="ExternalInput")
    sel_in = nc.dram_tensor("sel_in", [8, 4], f32, kind="ExternalInput")
    mats_in = nc.dram_tensor("mats_in", [128, 384], f32, kind="ExternalInput")

    mem_out = nc.dram_tensor("mem_out", [R, M], f32, kind="ExternalOutput")
    w_out = nc.dram_tensor("w_out", [R], f32, kind="ExternalOutput")

    # DRAM views
    mem_in_v = mem_in.ap().rearrange("(b j p) m -> b p j m", j=JB, p=128)
    mem_out_v = mem_out.ap().rearrange("(b j p) m -> b p j m", j=JB, p=128)
    pw_v = pw_in.ap().rearrange("(q s) -> q s", s=128)       # PM
    w_out_v = w_out.ap().rearrange("(q s) -> q s", s=128)    # PM

    rg = [[i for i in range(N_CORES)]]

    with tile.TileContext(nc) as tc:
        with (
            tc.tile_pool(name="cpool", bufs=1) as cpool,
            tc.tile_pool(name="rpool", bufs=1) as rpool,
            tc.tile_pool(name="spool", bufs=4) as spool,
            tc.tile_pool(name="ppool", bufs=1, space="PSUM") as ppool,
            tc.tile_pool(name="dpool", bufs=1, space="DRAM") as dpool,
        ):
            _emit(nc, tc, cpool, rpool, spool, ppool, dpool,
                  R, NT, JB, NB, AluOp, Act, rg, stage,
                  mem_in_v, mem_out_v, pw_v, w_out_v,
                  k_in, e_in, a_in, g_in, s_in, gam_in, sel_in, mats_in)

    nc.compile()
    return nc


def _emit(nc, tc, cpool, rpool, spool, ppool, dpool,
          R, NT, JB, NB, AluOp, Act, rg, stage,
          mem_in_v, mem_out_v, pw_v, w_out_v,
          k_in, e_in, a_in, g_in, s_in, gam_in, sel_in, mats_in):
    # ---------- start barrier ----------
    # The 8 cores launch with tens-of-us dispatch skew, which otherwise
    # surfaces as a wait inside collective 1 (after phase A, where nothing
    # can hide it). A no-payload AllGather up front synchronizes the cores
    # while DMA-in streams, gating only phase-A *compute* -- the barrier
    # wait overlaps the memory load almost entirely.
    pay0 = cpool.tile([1, 8], f32)
    nc.vector.memset(pay0[:], 0.0)
    cc0_in = dpool.tile([1, 8], f32)
    cc0_out = dpool.tile([8, 8], f32, addr_space="Shared")
    nc.gpsimd.dma_start(cc0_in[:], pay0[:])
    barrier = nc.gpsimd.collective_compute(
        "AllGather", AluOp.bypass, replica_groups=rg,
        ins=[cc0_in.opt()], outs=[cc0_out.opt()])

    # ---------- k chain first: k_b gates every dots op ----------
    k_row = cpool.tile([1, M], f32)
    nc.sync.dma_start(k_row[:], k_in.ap()[None, :])
    k_t = cpool.tile([1, M], f32)
    nc.scalar.activation(k_t[:], k_row[:], Act.Tanh)
    k_b = cpool.tile([128, M], f32)
    nc.gpsimd.partition_broadcast(k_b[:], k_t[:])

    # ---------- memory loads: alternate HWDGE (sync) / SWDGE (gpsimd)
    # descriptor generators -- one generator alone paces 1-KiB-descriptor
    # transfers at ~3.5us per MiB and becomes the phase-A limit ----------
    res = []
    for b in range(NB):
        rb = rpool.tile([128, JB * M], f32, tag=f"res{b}", name=f"res{b}")
        res.append(rb)
        eng = nc.sync if b % 2 == 0 else nc.gpsimd
        eng.dma_start(rb.rearrange("p (j m) -> p j m", m=M), mem_in_v[b])

    # ---------- phase A: row dots + row sumsq ----------
    dots_tm = cpool.tile([128, NT], f32)
    ssq_tm = cpool.tile([128, NT], f32)
    last_dve = last_act = None
    for b in range(NB):
        rb = res[b]
        for j in range(JB):
            i = b * JB + j
            sl = rb[:, j * M:(j + 1) * M]
            # dots via stt+accum: out = (mem * 1) .* k, accum = row sum.
            # (InstTensorTensorReduce faults the exec unit on this HW.)
            ttr_scr = spool.tile([128, M], f32, tag="ttr_scr",
                                 name="ttr_scr")
            cur = nc.vector.scalar_tensor_tensor(
                ttr_scr[:], sl, 1.0, k_b[:], AluOp.mult, AluOp.mult,
                accum_out=dots_tm[:, i:i + 1])
            if last_dve is None:
                _add_dep_helper(cur.ins, barrier.ins, sync=True,
                                reason="phase A compute after start barrier")
            last_dve = _chain(last_dve, cur)
            if i % SQ_DVE_EVERY == 2:
                sq_scr = spool.tile([128, M], f32, tag="sq_scr",
                                    name="sq_scr")
                last_dve = _chain(last_dve, nc.vector.scalar_tensor_tensor(
                    sq_scr[:], sl, 1.0, sl, AluOp.mult, AluOp.mult,
                    accum_out=ssq_tm[:, i:i + 1]))
            else:
                act_scr = spool.tile([128, M], f32, tag="act_scr",
                                     name="act_scr")
                cur = nc.scalar.activation(
                    act_scr[:], sl, Act.Square,
                    accum_out=ssq_tm[:, i:i + 1])
                if last_act is None:
                    _add_dep_helper(cur.ins, barrier.ins, sync=True,
                                    reason="phase A compute after barrier")
                last_act = _chain(last_act, cur)

    # ---------- remaining constants (scheduled below phase A priority) ----
    e_row = cpool.tile([1, M], f32)
    a_row = cpool.tile([1, M], f32)
    g_sb = cpool.tile([1, 1], f32)
    s_row = cpool.tile([1, 3], f32)
    gam_sb = cpool.tile([1, 1], f32)
    sel_sb = cpool.tile([8, 4], f32)
    pw_pm = cpool.tile([NT, 128], f32)
    mats_sb = cpool.tile([128, 384], f32)
    nc.sync.dma_start(e_row[:], e_in.ap()[None, :])
    nc.sync.dma_start(a_row[:], a_in.ap()[None, :])
    nc.sync.dma_start(g_sb[:], g_in.ap()[None, :])
    nc.sync.dma_start(s_row[:], s_in.ap()[None, :])
    nc.sync.dma_start(gam_sb[:], gam_in.ap()[None, :])
    nc.sync.dma_start(sel_sb[:], sel_in.ap())
    nc.sync.dma_start(pw_pm[:], pw_v)
    nc.sync.dma_start(mats_sb[:], mats_in.ap())
    identity = mats_sb[:, 0:128]
    shd = mats_sb[0:NT, 128:128 + NT]
    shu = mats_sb[0:NT, 256:256 + NT]

    k_sq = cpool.tile([1, M], f32)
    k_ss = cpool.tile([1, 1], f32)
    nc.scalar.activation(k_sq[:], k_t[:], Act.Square, accum_out=k_ss[:])
    k_nrm = cpool.tile([1, 1], f32)
    nc.scalar.activation(k_nrm[:], k_ss[:], Act.Sqrt)
    nc.vector.tensor_scalar_max(k_nrm[:], k_nrm[:], EPS)
    rk = cpool.tile([1, 1], f32)
    nc.vector.reciprocal(rk[:], k_nrm[:])
    rk_b = cpool.tile([128, 1], f32)
    nc.gpsimd.partition_broadcast(rk_b[:], rk[:])
    e_b = cpool.tile([128, M], f32)
    nc.gpsimd.partition_broadcast(e_b[:], e_row[:])
    a_b = cpool.tile([128, M], f32)
    nc.gpsimd.partition_broadcast(a_b[:], a_row[:])

    g_sg = cpool.tile([1, 1], f32)
    nc.scalar.activation(g_sg[:], g_sb[:], Act.Sigmoid)
    s_exp = cpool.tile([1, 3], f32)
    s_sum = cpool.tile([1, 1], f32)
    nc.scalar.activation(s_exp[:], s_row[:], Act.Exp, accum_out=s_sum[:])
    s_rs = cpool.tile([1, 1], f32)
    nc.vector.reciprocal(s_rs[:], s_sum[:])
    s_sm = cpool.tile([1, 3], f32)
    nc.vector.tensor_scalar(s_sm[:], s_exp[:], s_rs[:], None, AluOp.mult)
    s_b = cpool.tile([128, 3], f32)
    nc.gpsimd.partition_broadcast(s_b[:], s_sm[:])
    gam1 = cpool.tile([1, 1], f32)
    nc.scalar.activation(gam1[:], gam_sb[:], Act.Relu)
    nc.vector.tensor_scalar_add(gam1[:], gam1[:], 1.0)
    gam_b = cpool.tile([128, 1], f32)
    nc.gpsimd.partition_broadcast(gam_b[:], gam1[:])

    if stage < 2:
        _fallback_outputs(nc, res, mem_out_v, w_out_v, pw_pm, NB)
        return

    # ---------- phase B: cosine softmax (local part) ----------
    norm_tm = cpool.tile([128, NT], f32)
    nc.scalar.activation(norm_tm[:], ssq_tm[:], Act.Sqrt)
    nc.vector.tensor_scalar_max(norm_tm[:], norm_tm[:], EPS)
    rnorm_tm = cpool.tile([128, NT], f32)
    nc.vector.reciprocal(rnorm_tm[:], norm_tm[:])
    logit_tm = cpool.tile([128, NT], f32)
    nc.vector.tensor_mul(logit_tm[:], dots_tm[:], rnorm_tm[:])
    expx_tm = cpool.tile([128, NT], f32)
    s1_col = cpool.tile([128, 1], f32)
    nc.scalar.activation(expx_tm[:], logit_tm[:], Act.Exp,
                         scale=rk_b[:], accum_out=s1_col[:])
    s1_red = cpool.tile([128, 1], f32)
    nc.gpsimd.partition_all_reduce(s1_red[:], s1_col[:], 128,
                                   bass.bass_isa.ReduceOp.add)
    # warm the Ln/Exp activation tables while waiting on collective 1 --
    # ACT is idle in that window, so the ~1.3us table loads are free here
    # instead of landing on the post-AG1 serial chain.
    warm1 = cpool.tile([1, 1], f32)
    nc.scalar.activation(warm1[:], norm_tm[0:1, 0:1], Act.Ln)
    warm2 = cpool.tile([1, 1], f32)
    nc.scalar.activation(warm2[:], norm_tm[0:1, 0:1], Act.Exp)

    # payload: [s1, exp_first, exp_last, pw_first, pw_last, 0, 0, 0]
    pay1 = cpool.tile([1, 8], f32)
    nc.vector.memset(pay1[:], 0.0)
    nc.vector.tensor_copy(pay1[:, 0:1], s1_red[0:1, 0:1])
    nc.vector.tensor_copy(pay1[:, 1:2], expx_tm[0:1, 0:1])
    nc.gpsimd.dma_start(pay1[:, 2:3], expx_tm[127:128, NT - 1:NT])
    nc.vector.tensor_copy(pay1[:, 3:4], pw_pm[0:1, 0:1])
    nc.gpsimd.dma_start(pay1[:, 4:5], pw_pm[NT - 1:NT, 127:128])

    cc1_in = dpool.tile([1, 8], f32)
    cc1_out = dpool.tile([8, 8], f32, addr_space="Shared")
    nc.gpsimd.dma_start(cc1_in[:], pay1[:])
    nc.gpsimd.collective_compute(
        "AllGather", AluOp.bypass, replica_groups=rg,
        ins=[cc1_in.opt()], outs=[cc1_out.opt()])
    ag1 = cpool.tile([8, 8], f32)
    nc.gpsimd.dma_start(ag1[:], cc1_out[:])

    # selector matmuls: all land on partition 0
    psumA = ppool.tile([1, 8], f32, tag="psumA", name="psumA")
    nc.tensor.matmul(psumA[:], sel_sb[:, 2:3], ag1[:])
    psumL = ppool.tile([1, 8], f32, tag="psumL", name="psumL")
    nc.tensor.matmul(psumL[:], sel_sb[:, 0:1], ag1[:])
    psumR = ppool.tile([1, 8], f32, tag="psumR", name="psumR")
    nc.tensor.matmul(psumR[:], sel_sb[:, 1:2], ag1[:])

    rS1 = cpool.tile([1, 1], f32)
    nc.vector.reciprocal(rS1[:], psumA[0:1, 0:1])
    go2 = cpool.tile([1, 2], f32)      # [g/S1, 1-g] in one row
    nc.vector.tensor_mul(go2[:, 0:1], g_sg[:], rS1[:])
    nc.vector.tensor_scalar(go2[:, 1:2], g_sg[:], -1.0, 1.0,
                            AluOp.mult, AluOp.add)
    go2_b = cpool.tile([128, 2], f32)
    nc.gpsimd.partition_broadcast(go2_b[:], go2[:])
    gS1 = go2[:, 0:1]
    omg = go2[:, 1:2]
    gS1_b = go2_b[:, 0:1]
    omg_b = go2_b[:, 1:2]

    # halo wg values (both computed on p0):
    #   left -> wg_ext[0,0] (copy), right -> wg_ext[NT-1,129] (dma)
    wg_ext = cpool.tile([NT, 130], f32)
    hx = cpool.tile([1, 1], f32)
    hy = cpool.tile([1, 1], f32)
    hl = cpool.tile([1, 1], f32)
    nc.vector.tensor_mul(hx[:], psumL[0:1, 2:3], gS1)
    nc.vector.tensor_mul(hy[:], psumL[0:1, 4:5], omg)
    nc.vector.tensor_add(hl[:], hx[:], hy[:])
    hrx = cpool.tile([1, 1], f32)
    hry = cpool.tile([1, 1], f32)
    hr = cpool.tile([1, 1], f32)
    nc.vector.tensor_mul(hrx[:], psumR[0:1, 1:2], gS1)
    nc.vector.tensor_mul(hry[:], psumR[0:1, 3:4], omg)
    nc.vector.tensor_add(hr[:], hrx[:], hry[:])

    if stage < 3:
        _fallback_outputs(nc, res, mem_out_v, w_out_v, pw_pm, NB)
        return

    # wg in PM layout
    exp_pm = ppool.tile([NT, 128], f32, tag="exp_pm", name="exp_pm")
    nc.tensor.transpose(exp_pm[:], expx_tm[:], identity[:])
    wgt2 = cpool.tile([NT, 128], f32)
    nc.vector.tensor_scalar(wgt2[:], pw_pm[:], omg_b[0:NT, :], None,
                            AluOp.mult)
    nc.vector.scalar_tensor_tensor(wg_ext[:, 1:129], exp_pm[:],
                                   gS1_b[0:NT, :], wgt2[:],
                                   AluOp.mult, AluOp.add)

    # roll carries via PE shift matrices (cross-partition moves);
    # full-column copies first, then the cross-core halos overlay
    # the [0,0] / [NT-1,129] corners.
    carryD = ppool.tile([NT, 1], f32, tag="carryD", name="carryD")
    nc.tensor.matmul(carryD[:], shd[:], wg_ext[:, 128:129])
    nc.vector.tensor_copy(wg_ext[:, 0:1], carryD[:])
    carryU = ppool.tile([NT, 1], f32, tag="carryU", name="carryU")
    nc.tensor.matmul(carryU[:], shu[:], wg_ext[:, 1:2])
    nc.vector.tensor_copy(wg_ext[:, 129:130], carryU[:])
    nc.vector.tensor_copy(wg_ext[0:1, 0:1], hl[:])
    nc.gpsimd.dma_start(wg_ext[NT - 1:NT, 129:130], hr[:])

    # ws = s0*wg[-1] + s1*wg + s2*wg[+1]  (PM, fused stt chain)
    ws_t0 = cpool.tile([NT, 128], f32)
    nc.vector.tensor_scalar(ws_t0[:], wg_ext[0:NT, 0:128],
                            s_b[0:NT, 0:1], None, AluOp.mult)
    ws_t1 = cpool.tile([NT, 128], f32)
    nc.vector.scalar_tensor_tensor(ws_t1[:], wg_ext[0:NT, 1:129],
                                   s_b[0:NT, 1:2], ws_t0[:],
                                   AluOp.mult, AluOp.add)
    ws_pm = cpool.tile([NT, 128], f32)
    nc.vector.scalar_tensor_tensor(ws_pm[:], wg_ext[0:NT, 2:130],
                                   s_b[0:NT, 2:3], ws_t1[:],
                                   AluOp.mult, AluOp.add)

    # u = ws ** gamma = exp(gamma * ln(ws)); local sum via accum
    ln_ws = cpool.tile([NT, 128], f32)
    nc.scalar.activation(ln_ws[:], ws_pm[:], Act.Ln)
    gl = cpool.tile([NT, 128], f32)
    nc.vector.tensor_scalar(gl[:], ln_ws[:], gam_b[0:NT, :], None,
                            AluOp.mult)
    u_pm = cpool.tile([NT, 128], f32)
    u_acc = cpool.tile([NT, 1], f32)
    nc.scalar.activation(u_pm[:], gl[:], Act.Exp, accum_out=u_acc[:])
    s2_red = cpool.tile([NT, 1], f32)
    nc.gpsimd.partition_all_reduce(s2_red[:], u_acc[:], NT,
                                   bass.bass_isa.ReduceOp.add)

    pay2 = cpool.tile([1, 8], f32)
    nc.vector.memset(pay2[:], 0.0)
    nc.vector.tensor_copy(pay2[:, 0:1], s2_red[0:1, 0:1])
    cc2_in = dpool.tile([1, 8], f32)
    cc2_out = dpool.tile([8, 8], f32, addr_space="Shared")
    nc.gpsimd.dma_start(cc2_in[:], pay2[:])
    nc.gpsimd.collective_compute(
        "AllGather", AluOp.bypass, replica_groups=rg,
        ins=[cc2_in.opt()], outs=[cc2_out.opt()])
    ag2 = cpool.tile([8, 8], f32)
    nc.gpsimd.dma_start(ag2[:], cc2_out[:])
    psumC = ppool.tile([1, 8], f32, tag="psumC", name="psumC")
    nc.tensor.matmul(psumC[:], sel_sb[:, 2:3], ag2[:])
    rS2 = cpool.tile([1, 1], f32)
    nc.vector.reciprocal(rS2[:], psumC[0:1, 0:1])
    rS2_b = cpool.tile([128, 1], f32)
    nc.gpsimd.partition_broadcast(rS2_b[:], rS2[:])

    if stage < 4:
        _fallback_outputs(nc, res, mem_out_v, w_out_v, pw_pm, NB)
        return

    w_pm = cpool.tile([NT, 128], f32)
    nc.vector.tensor_scalar(w_pm[:], u_pm[:], rS2_b[0:NT, :], None,
                            AluOp.mult)
    nc.sync.dma_start(w_out_v, w_pm[:])

    # w back to TM for per-sub-tile column scalars
    w_tm_ps = ppool.tile([128, NT], f32, tag="w_tm_ps", name="w_tm_ps")
    nc.tensor.transpose(w_tm_ps[:], w_pm[:], identity[0:NT, 0:NT])
    w_tm = cpool.tile([128, NT], f32)
    nc.scalar.copy(w_tm[:], w_tm_ps[:])
    w_ng = cpool.tile([128, NT], f32)
    nc.vector.tensor_scalar(w_ng[:], w_tm_ps[:], -1.0, None, AluOp.mult)

    # ---------- update: new = mem*(1 - w e) + w a ----------
    # two passes per big tile (all P1+mul first, adds second) so the DVE
    # adds don't head-of-line-block behind individual GPSIMD muls
    last_gp = last_dve2 = last_act2 = None
    for b in range(NB):
        rb = res[b]
        tiles = []
        for j in range(JB):
            i = b * JB + j
            sl = rb[:, j * M:(j + 1) * M]
            p1_scr = spool.tile([128, M], f32, tag="p1_scr", name="p1_scr",
                                bufs=12)
            last_act2 = _chain(last_act2, nc.scalar.activation(
                p1_scr[:], e_b[:], Act.Copy,
                bias=1.0, scale=w_ng[:, i:i + 1]))
            # erase-mul: mostly GPSIMD tt; every Nth on DVE as stt (DVE's
            # plain tt runs ~2x slower than stt on this HW). Separate
            # scratch tags per engine -- a shared tag's slot rotation would
            # serialize the fast engine behind the slow one.
            if i % MUL_DVE_EVERY == 0:
                t_scr = spool.tile([128, M], f32, tag="t_scr_dve",
                                   name="t_scr_dve", bufs=6)
                last_dve2 = _chain(last_dve2, nc.vector.scalar_tensor_tensor(
                    t_scr[:], p1_scr[:], 1.0, sl, AluOp.mult, AluOp.mult))
            else:
                t_scr = spool.tile([128, M], f32, tag="t_scr_gp",
                                   name="t_scr_gp", bufs=12)
                last_gp = _chain(last_gp, nc.gpsimd.tensor_mul(
                    t_scr[:], sl, p1_scr[:]))
            tiles.append((i, sl, t_scr))
        for i, sl, t_scr in tiles:
            last_dve2 = _chain(last_dve2, nc.vector.scalar_tensor_tensor(
                sl, a_b[:], w_tm[:, i:i + 1],
                t_scr[:], AluOp.mult, AluOp.add))
        nc.sync.dma_start(mem_out_v[b],
                          rb.rearrange("p (j m) -> p j m", m=M))


def _fallback_outputs(nc, res, mem_out_v, w_out_v, pw_pm, NB):
    """Truncated-pipeline stand-in writes so every output is produced."""
    for b in range(NB):
        nc.sync.dma_start(mem_out_v[b],
                          res[b].rearrange("p (j m) -> p j m", m=M))
    nc.sync.dma_start(w_out_v, pw_pm[:])


_NC_CACHE = {}


def _get_nc(R, stage=4):
    key = (R, stage)
    if key not in _NC_CACHE:
        _NC_CACHE[key] = build_nc(R, stage)
    return _NC_CACHE[key]


def _sel_matrix(c):
    sel = np.zeros((8, 4), dtype=np.float32)
    sel[(c - 1) % N_CORES, 0] = 1.0   # left neighbor
    sel[(c + 1) % N_CORES, 1] = 1.0   # right neighbor
    sel[:, 2] = 1.0                   # sums
    return sel


def _mats():
    """[identity | shift-down | shift-up] packed as one (128, 384) input."""
    m = np.zeros((128, 384), dtype=np.float32)
    m[:, 0:128] = np.eye(128, dtype=np.float32)
    m[:, 128:256] = np.eye(128, k=1, dtype=np.float32)   # shd[k,m]=1, k==m-1
    m[:, 256:384] = np.eye(128, k=-1, dtype=np.float32)  # shu[k,m]=1, k==m+1
    return m


def _in_maps(memory, k, g, s, gamma, prev_w, e, a):
    N = memory.shape[0]
    R = N // N_CORES
    f4 = np.float32
    base = {
        "k_in": np.ascontiguousarray(k, dtype=f4),
        "e_in": np.ascontiguousarray(e, dtype=f4),
        "a_in": np.ascontiguousarray(a, dtype=f4),
        "g_in": np.ascontiguousarray(g, dtype=f4),
        "s_in": np.ascontiguousarray(s, dtype=f4),
        "gam_in": np.ascontiguousarray(gamma, dtype=f4),
    }
    maps = []
    for c in range(N_CORES):
        m = dict(base)
        m["mem_in"] = np.ascontiguousarray(memory[c * R:(c + 1) * R],
                                           dtype=f4)
        m["pw_in"] = np.ascontiguousarray(prev_w[c * R:(c + 1) * R],
                                          dtype=f4)
        m["sel_in"] = _sel_matrix(c)
        m["mats_in"] = _mats()
        maps.append(m)
    return R, maps


def run(memory, k, g, s, gamma, prev_w, e, a, stage=4, trace=False):
    R, in_maps = _in_maps(memory, k, g, s, gamma, prev_w, e, a)
    nc = _get_nc(R, stage)
    out = bass_utils.run_bass_kernel_spmd(nc, in_maps,
                                          core_ids=list(range(N_CORES)),
                                          trace=trace)
    new_memory = np.concatenate([out.results[c]["mem_out"]
                                 for c in range(N_CORES)], axis=0)
    w = np.concatenate([out.results[c]["w_out"]
                        for c in range(N_CORES)], axis=0)
    return new_memory, w


def kernel(memory, k, beta, g, s, gamma, prev_w, e, a):
    return run(np.asarray(memory), np.asarray(k), np.asarray(g),
               np.asarray(s), np.asarray(gamma), np.asarray(prev_w),
               np.asarray(e), np.asarray(a))


# revision 20
# speedup vs baseline: 1.1122x; 1.1122x over previous
"""NTM-style scatter_memory kernel for Trainium2, 8 NeuronCores.

Reference math (per problem nn_Memory_72249939853591):
    k = tanh(k); g = sigmoid(g); s = softmax(s); gamma = relu(gamma) + 1
    dots = memory @ k
    wc = softmax(dots / (max(|row|,eps) * max(|k|,eps)))
    wg = g * wc + (1 - g) * prev_w
    ws = s0 * roll(wg, 1) + s1 * wg + s2 * roll(wg, -1)
    w = ws ** gamma; w /= sum(w)
    new_memory = memory * (1 - w e^T) + w a^T
    returns (new_memory, w)

Sharding: memory / prev_w / w row-sharded over 8 cores (16384 rows each).
Cross-core communication: two tiny AllGathers (softmax denominator +
1-element roll halos, then the sharpening normalizer).

Per-core layouts:
  TM (tile-major):  (128, NT) tile t[p, i] = vec[i*128 + p]   -- matches
      how (128,256) memory sub-tiles land in SBUF partitions.
  PM (part-major):  (NT, 128) tile t[q, s] = vec[q*128 + s]   -- the roll
      is a free-axis shift here; carries cross partitions via tiny PE
      shift-matrix matmuls. TM <-> PM is a PE transpose.
"""

import numpy as np

import concourse.bacc as bacc
import concourse.bass as bass
import concourse.mybir as mybir
import concourse.tile as tile
from concourse import bass_utils, masks
from concourse.bass import _add_dep_helper


def _chain(prev, cur):
    """Order-only dep: keep same-engine streams in emission order so the
    tile scheduler can't round-robin them against DMA arrival order."""
    if prev is not None:
        _add_dep_helper(cur.ins, prev.ins, sync=False,
                        reason="keep engine stream in emission order")
    return cur

f32 = mybir.dt.float32
N_CORES = 8
M = 256
EPS = 1e-8
# engine balance knobs (measured costs: ACT sq+acc 814ns, DVE stt+acc 528ns,
# DVE stt 446ns, GP tt 766ns, ACT P1 535ns per 128x256 f32 tile)
SQ_DVE_EVERY = 5   # every 5th sub-tile's row-sumsq on DVE instead of ACT
MUL_DVE_EVERY = 12  # every 4th sub-tile's erase-mul on DVE instead of GPSIMD


def build_nc(R, stage=4):
    """Build + compile the per-core Bass module for R rows per core.

    stage < 4 truncates the pipeline for hardware hang bisection:
      1 = consts + phase A only (copy-through outputs)
      2 = + phase B, collective 1, selector matmuls, halo scalars
      3 = + transpose/wg/roll/sharpen, collective 2
      4 = full kernel
    """
    NT = R // 128          # number of 128-row sub-tiles
    JB = 8                 # sub-tiles per big DMA tile
    NB = NT // JB          # big tiles (1 MiB each)
    assert NB * JB == NT
    AluOp = mybir.AluOpType
    Act = mybir.ActivationFunctionType

    nc = bacc.Bacc("TRN2", target_bir_lowering=False, debug=False,
                   num_devices=N_CORES)

    mem_in = nc.dram_tensor("mem_in", [R, M], f32, kind="ExternalInput")
    pw_in = nc.dram_tensor("pw_in", [R], f32, kind="ExternalInput")
    k_in = nc.dram_tensor("k_in", [M], f32, kind="ExternalInput")
    e_in = nc.dram_tensor("e_in", [M], f32, kind="ExternalInput")
    a_in = nc.dram_tensor("a_in", [M], f32, kind="ExternalInput")
    g_in = nc.dram_tensor("g_in", [1], f32, kind="ExternalInput")
    s_in = nc.dram_tensor("s_in", [3], f32, kind="ExternalInput")
    gam_in = nc.dram_tensor("gam_in", [1], f32, kind="ExternalInput")
    sel_in = nc.dram_tensor("sel_in", [8, 4], f32, kind="ExternalInput")
    mats_in = nc.dram_tensor("mats_in", [128, 384], f32, kind="ExternalInput")

    mem_out = nc.dram_tensor("mem_out", [R, M], f32, kind="ExternalOutput")
    w_out = nc.dram_tensor("w_out", [R], f32, kind="ExternalOutput")

    # DRAM views
    mem_in_v = mem_in.ap().rearrange("(b j p) m -> b p j m", j=JB, p=128)
    mem_out_v = mem_out.ap().rearrange("(b j p) m -> b p j m", j=JB, p=128)
    pw_v = pw_in.ap().rearrange("(q s) -> q s", s=128)       # PM
    w_out_v = w_out.ap().rearrange("(q s) -> q s", s=128)    # PM

    rg = [[i for i in range(N_CORES)]]

    with tile.TileContext(nc) as tc:
        with (
            tc.tile_pool(name="cpool", bufs=1) as cpool,
            tc.tile_pool(name="rpool", bufs=1) as rpool,
            tc.tile_pool(name="spool", bufs=4) as spool,
            tc.tile_pool(name="ppool", bufs=1, space="PSUM") as ppool,
            tc.tile_pool(name="dpool", bufs=1, space="DRAM") as dpool,
        ):
            _emit(nc, tc, cpool, rpool, spool, ppool, dpool,
                  R, NT, JB, NB, AluOp, Act, rg, stage,
                  mem_in_v, mem_out_v, pw_v, w_out_v,
                  k_in, e_in, a_in, g_in, s_in, gam_in, sel_in, mats_in)

    nc.compile()
    return nc


def _emit(nc, tc, cpool, rpool, spool, ppool, dpool,
          R, NT, JB, NB, AluOp, Act, rg, stage,
          mem_in_v, mem_out_v, pw_v, w_out_v,
          k_in, e_in, a_in, g_in, s_in, gam_in, sel_in, mats_in):
    # ---------- k chain first: k_b gates every dots op ----------
    k_row = cpool.tile([1, M], f32)
    nc.sync.dma_start(k_row[:], k_in.ap()[None, :])
    k_t = cpool.tile([1, M], f32)
    nc.scalar.activation(k_t[:], k_row[:], Act.Tanh)
    k_b = cpool.tile([128, M], f32)
    nc.gpsimd.partition_broadcast(k_b[:], k_t[:])

    # ---------- memory loads: alternate HWDGE (sync) / SWDGE (gpsimd)
    # descriptor generators -- one generator alone paces 1-KiB-descriptor
    # transfers at ~3.5us per MiB and becomes the phase-A limit ----------
    res = []
    for b in range(NB):
        rb = rpool.tile([128, JB * M], f32, tag=f"res{b}", name=f"res{b}")
        res.append(rb)
        eng = nc.sync if b % 2 == 0 else nc.gpsimd
        eng.dma_start(rb.rearrange("p (j m) -> p j m", m=M), mem_in_v[b])

    # ---------- phase A: row dots + row sumsq ----------
    dots_tm = cpool.tile([128, NT], f32)
    ssq_tm = cpool.tile([128, NT], f32)
    last_dve = last_act = None
    for b in range(NB):
        rb = res[b]
        for j in range(JB):
            i = b * JB + j
            sl = rb[:, j * M:(j + 1) * M]
            # dots via stt+accum: out = (mem * 1) .* k, accum = row sum.
            # (InstTensorTensorReduce faults the exec unit on this HW.)
            ttr_scr = spool.tile([128, M], f32, tag="ttr_scr",
                                 name="ttr_scr")
            last_dve = _chain(last_dve, nc.vector.scalar_tensor_tensor(
                ttr_scr[:], sl, 1.0, k_b[:], AluOp.mult, AluOp.mult,
                accum_out=dots_tm[:, i:i + 1]))
            if i % SQ_DVE_EVERY == 2:
                sq_scr = spool.tile([128, M], f32, tag="sq_scr",
                                    name="sq_scr")
                last_dve = _chain(last_dve, nc.vector.scalar_tensor_tensor(
                    sq_scr[:], sl, 1.0, sl, AluOp.mult, AluOp.mult,
                    accum_out=ssq_tm[:, i:i + 1]))
            else:
                act_scr = spool.tile([128, M], f32, tag="act_scr",
                                     name="act_scr")
                last_act = _chain(last_act, nc.scalar.activation(
                    act_scr[:], sl, Act.Square,
                    accum_out=ssq_tm[:, i:i + 1]))

    # ---------- remaining constants (scheduled below phase A priority) ----
    e_row = cpool.tile([1, M], f32)
    a_row = cpool.tile([1, M], f32)
    g_sb = cpool.tile([1, 1], f32)
    s_row = cpool.tile([1, 3], f32)
    gam_sb = cpool.tile([1, 1], f32)
    sel_sb = cpool.tile([8, 4], f32)
    pw_pm = cpool.tile([NT, 128], f32)
    mats_sb = cpool.tile([128, 384], f32)
    nc.sync.dma_start(e_row[:], e_in.ap()[None, :])
    nc.sync.dma_start(a_row[:], a_in.ap()[None, :])
    nc.sync.dma_start(g_sb[:], g_in.ap()[None, :])
    nc.sync.dma_start(s_row[:], s_in.ap()[None, :])
    nc.sync.dma_start(gam_sb[:], gam_in.ap()[None, :])
    nc.sync.dma_start(sel_sb[:], sel_in.ap())
    nc.sync.dma_start(pw_pm[:], pw_v)
    nc.sync.dma_start(mats_sb[:], mats_in.ap())
    identity = mats_sb[:, 0:128]
    shd = mats_sb[0:NT, 128:128 + NT]
    shu = mats_sb[0:NT, 256:256 + NT]

    k_sq = cpool.tile([1, M], f32)
    k_ss = cpool.tile([1, 1], f32)
    nc.scalar.activation(k_sq[:], k_t[:], Act.Square, accum_out=k_ss[:])
    k_nrm = cpool.tile([1, 1], f32)
    nc.scalar.activation(k_nrm[:], k_ss[:], Act.Sqrt)
    nc.vector.tensor_scalar_max(k_nrm[:], k_nrm[:], EPS)
    rk = cpool.tile([1, 1], f32)
    nc.vector.reciprocal(rk[:], k_nrm[:])
    rk_b = cpool.tile([128, 1], f32)
    nc.gpsimd.partition_broadcast(rk_b[:], rk[:])
    e_b = cpool.tile([128, M], f32)
    nc.gpsimd.partition_broadcast(e_b[:], e_row[:])
    a_b = cpool.tile([128, M], f32)
    nc.gpsimd.partition_broadcast(a_b[:], a_row[:])

    g_sg = cpool.tile([1, 1], f32)
    nc.scalar.activation(g_sg[:], g_sb[:], Act.Sigmoid)
    s_exp = cpool.tile([1, 3], f32)
    s_sum = cpool.tile([1, 1], f32)
    nc.scalar.activation(s_exp[:], s_row[:], Act.Exp, accum_out=s_sum[:])
    s_rs = cpool.tile([1, 1], f32)
    nc.vector.reciprocal(s_rs[:], s_sum[:])
    s_sm = cpool.tile([1, 3], f32)
    nc.vector.tensor_scalar(s_sm[:], s_exp[:], s_rs[:], None, AluOp.mult)
    s_b = cpool.tile([128, 3], f32)
    nc.gpsimd.partition_broadcast(s_b[:], s_sm[:])
    gam1 = cpool.tile([1, 1], f32)
    nc.scalar.activation(gam1[:], gam_sb[:], Act.Relu)
    nc.vector.tensor_scalar_add(gam1[:], gam1[:], 1.0)
    gam_b = cpool.tile([128, 1], f32)
    nc.gpsimd.partition_broadcast(gam_b[:], gam1[:])

    if stage < 2:
        _fallback_outputs(nc, res, mem_out_v, w_out_v, pw_pm, NB)
        return

    # ---------- phase B: cosine softmax (local part) ----------
    norm_tm = cpool.tile([128, NT], f32)
    nc.scalar.activation(norm_tm[:], ssq_tm[:], Act.Sqrt)
    nc.vector.tensor_scalar_max(norm_tm[:], norm_tm[:], EPS)
    rnorm_tm = cpool.tile([128, NT], f32)
    nc.vector.reciprocal(rnorm_tm[:], norm_tm[:])
    logit_tm = cpool.tile([128, NT], f32)
    nc.vector.tensor_mul(logit_tm[:], dots_tm[:], rnorm_tm[:])
    expx_tm = cpool.tile([128, NT], f32)
    s1_col = cpool.tile([128, 1], f32)
    nc.scalar.activation(expx_tm[:], logit_tm[:], Act.Exp,
                         scale=rk_b[:], accum_out=s1_col[:])
    s1_red = cpool.tile([128, 1], f32)
    nc.gpsimd.partition_all_reduce(s1_red[:], s1_col[:], 128,
                                   bass.bass_isa.ReduceOp.add)
    # warm the Ln/Exp activation tables while waiting on collective 1 --
    # ACT is idle in that window, so the ~1.3us table loads are free here
    # instead of landing on the post-AG1 serial chain.
    warm1 = cpool.tile([1, 1], f32)
    nc.scalar.activation(warm1[:], norm_tm[0:1, 0:1], Act.Ln)
    warm2 = cpool.tile([1, 1], f32)
    nc.scalar.activation(warm2[:], norm_tm[0:1, 0:1], Act.Exp)

    # payload: [s1, exp_first, exp_last, pw_first, pw_last, 0, 0, 0]
    pay1 = cpool.tile([1, 8], f32)
    nc.vector.memset(pay1[:], 0.0)
    nc.vector.tensor_copy(pay1[:, 0:1], s1_red[0:1, 0:1])
    nc.vector.tensor_copy(pay1[:, 1:2], expx_tm[0:1, 0:1])
    nc.gpsimd.dma_start(pay1[:, 2:3], expx_tm[127:128, NT - 1:NT])
    nc.vector.tensor_copy(pay1[:, 3:4], pw_pm[0:1, 0:1])
    nc.gpsimd.dma_start(pay1[:, 4:5], pw_pm[NT - 1:NT, 127:128])

    cc1_in = dpool.tile([1, 8], f32)
    cc1_out = dpool.tile([8, 8], f32, addr_space="Shared")
    nc.gpsimd.dma_start(cc1_in[:], pay1[:])
    nc.gpsimd.collective_compute(
        "AllGather", AluOp.bypass, replica_groups=rg,
        ins=[cc1_in.opt()], outs=[cc1_out.opt()])
    ag1 = cpool.tile([8, 8], f32)
    nc.gpsimd.dma_start(ag1[:], cc1_out[:])

    # selector matmuls: all land on partition 0
    psumA = ppool.tile([1, 8], f32, tag="psumA", name="psumA")
    nc.tensor.matmul(psumA[:], sel_sb[:, 2:3], ag1[:])
    psumL = ppool.tile([1, 8], f32, tag="psumL", name="psumL")
    nc.tensor.matmul(psumL[:], sel_sb[:, 0:1], ag1[:])
    psumR = ppool.tile([1, 8], f32, tag="psumR", name="psumR")
    nc.tensor.matmul(psumR[:], sel_sb[:, 1:2], ag1[:])

    rS1 = cpool.tile([1, 1], f32)
    nc.vector.reciprocal(rS1[:], psumA[0:1, 0:1])
    go2 = cpool.tile([1, 2], f32)      # [g/S1, 1-g] in one row
    nc.vector.tensor_mul(go2[:, 0:1], g_sg[:], rS1[:])
    nc.vector.tensor_scalar(go2[:, 1:2], g_sg[:], -1.0, 1.0,
                            AluOp.mult, AluOp.add)
    go2_b = cpool.tile([128, 2], f32)
    nc.gpsimd.partition_broadcast(go2_b[:], go2[:])
    gS1 = go2[:, 0:1]
    omg = go2[:, 1:2]
    gS1_b = go2_b[:, 0:1]
    omg_b = go2_b[:, 1:2]

    # halo wg values (both computed on p0):
    #   left -> wg_ext[0,0] (copy), right -> wg_ext[NT-1,129] (dma)
    wg_ext = cpool.tile([NT, 130], f32)
    hx = cpool.tile([1, 1], f32)
    hy = cpool.tile([1, 1], f32)
    hl = cpool.tile([1, 1], f32)
    nc.vector.tensor_mul(hx[:], psumL[0:1, 2:3], gS1)
    nc.vector.tensor_mul(hy[:], psumL[0:1, 4:5], omg)
    nc.vector.tensor_add(hl[:], hx[:], hy[:])
    hrx = cpool.tile([1, 1], f32)
    hry = cpool.tile([1, 1], f32)
    hr = cpool.tile([1, 1], f32)
    nc.vector.tensor_mul(hrx[:], psumR[0:1, 1:2], gS1)
    nc.vector.tensor_mul(hry[:], psumR[0:1, 3:4], omg)
    nc.vector.tensor_add(hr[:], hrx[:], hry[:])

    if stage < 3:
        _fallback_outputs(nc, res, mem_out_v, w_out_v, pw_pm, NB)
        return

    # wg in PM layout
    exp_pm = ppool.tile([NT, 128], f32, tag="exp_pm", name="exp_pm")
    nc.tensor.transpose(exp_pm[:], expx_tm[:], identity[:])
    wgt2 = cpool.tile([NT, 128], f32)
    nc.vector.tensor_scalar(wgt2[:], pw_pm[:], omg_b[0:NT, :], None,
                            AluOp.mult)
    nc.vector.scalar_tensor_tensor(wg_ext[:, 1:129], exp_pm[:],
                                   gS1_b[0:NT, :], wgt2[:],
                                   AluOp.mult, AluOp.add)

    # roll carries via PE shift matrices (cross-partition moves);
    # full-column copies first, then the cross-core halos overlay
    # the [0,0] / [NT-1,129] corners.
    carryD = ppool.tile([NT, 1], f32, tag="carryD", name="carryD")
    nc.tensor.matmul(carryD[:], shd[:], wg_ext[:, 128:129])
    nc.vector.tensor_copy(wg_ext[:, 0:1], carryD[:])
    carryU = ppool.tile([NT, 1], f32, tag="carryU", name="carryU")
    nc.tensor.matmul(carryU[:], shu[:], wg_ext[:, 1:2])
    nc.vector.tensor_copy(wg_ext[:, 129:130], carryU[:])
    nc.vector.tensor_copy(wg_ext[0:1, 0:1], hl[:])
    nc.gpsimd.dma_start(wg_ext[NT - 1:NT, 129:130], hr[:])

    # ws = s0*wg[-1] + s1*wg + s2*wg[+1]  (PM, fused stt chain)
    ws_t0 = cpool.tile([NT, 128], f32)
    nc.vector.tensor_scalar(ws_t0[:], wg_ext[0:NT, 0:128],
                            s_b[0:NT, 0:1], None, AluOp.mult)
    ws_t1 = cpool.tile([NT, 128], f32)
    nc.vector.scalar_tensor_tensor(ws_t1[:], wg_ext[0:NT, 1:129],
                                   s_b[0:NT, 1:2], ws_t0[:],
                                   AluOp.mult, AluOp.add)
    ws_pm = cpool.tile([NT, 128], f32)
    nc.vector.scalar_tensor_tensor(ws_pm[:], wg_ext[0:NT, 2:130],
                                   s_b[0:NT, 2:3], ws_t1[:],
                                   AluOp.mult, AluOp.add)

    # u = ws ** gamma = exp(gamma * ln(ws)); local sum via accum
    ln_ws = cpool.tile([NT, 128], f32)
    nc.scalar.activation(ln_ws[:], ws_pm[:], Act.Ln)
    gl = cpool.tile([NT, 128], f32)
    nc.vector.tensor_scalar(gl[:], ln_ws[:], gam_b[0:NT, :], None,
                            AluOp.mult)
    u_pm = cpool.tile([NT, 128], f32)
    u_acc = cpool.tile([NT, 1], f32)
    nc.scalar.activation(u_pm[:], gl[:], Act.Exp, accum_out=u_acc[:])
    s2_red = cpool.tile([NT, 1], f32)
    nc.gpsimd.partition_all_reduce(s2_red[:], u_acc[:], NT,
                                   bass.bass_isa.ReduceOp.add)

    pay2 = cpool.tile([1, 8], f32)
    nc.vector.memset(pay2[:], 0.0)
    nc.vector.tensor_copy(pay2[:, 0:1], s2_red[0:1, 0:1])
    cc2_in = dpool.tile([1, 8], f32)
    cc2_out = dpool.tile([8, 8], f32, addr_space="Shared")
    nc.gpsimd.dma_start(cc2_in[:], pay2[:])
    nc.gpsimd.collective_compute(
        "AllGather", AluOp.bypass, replica_groups=rg,
        ins=[cc2_in.opt()], outs=[cc2_out.opt()])
    ag2 = cpool.tile([8, 8], f32)
    nc.gpsimd.dma_start(ag2[:], cc2_out[:])
    psumC = ppool.tile([1, 8], f32, tag="psumC", name="psumC")
    nc.tensor.matmul(psumC[:], sel_sb[:, 2:3], ag2[:])
    rS2 = cpool.tile([1, 1], f32)
    nc.vector.reciprocal(rS2[:], psumC[0:1, 0:1])
    rS2_b = cpool.tile([128, 1], f32)
    nc.gpsimd.partition_broadcast(rS2_b[:], rS2[:])

    if stage < 4:
        _fallback_outputs(nc, res, mem_out_v, w_out_v, pw_pm, NB)
        return

    w_pm = cpool.tile([NT, 128], f32)
    nc.vector.tensor_scalar(w_pm[:], u_pm[:], rS2_b[0:NT, :], None,
                            AluOp.mult)
    nc.sync.dma_start(w_out_v, w_pm[:])

    # w back to TM for per-sub-tile column scalars
    w_tm_ps = ppool.tile([128, NT], f32, tag="w_tm_ps", name="w_tm_ps")
    nc.tensor.transpose(w_tm_ps[:], w_pm[:], identity[0:NT, 0:NT])
    w_tm = cpool.tile([128, NT], f32)
    nc.scalar.copy(w_tm[:], w_tm_ps[:])
    w_ng = cpool.tile([128, NT], f32)
    nc.vector.tensor_scalar(w_ng[:], w_tm_ps[:], -1.0, None, AluOp.mult)

    # ---------- update: new = mem*(1 - w e) + w a ----------
    # two passes per big tile (all P1+mul first, adds second) so the DVE
    # adds don't head-of-line-block behind individual GPSIMD muls
    last_gp = last_dve2 = last_act2 = None
    for b in range(NB):
        rb = res[b]
        tiles = []
        for j in range(JB):
            i = b * JB + j
            sl = rb[:, j * M:(j + 1) * M]
            p1_scr = spool.tile([128, M], f32, tag="p1_scr", name="p1_scr",
                                bufs=12)
            last_act2 = _chain(last_act2, nc.scalar.activation(
                p1_scr[:], e_b[:], Act.Copy,
                bias=1.0, scale=w_ng[:, i:i + 1]))
            # erase-mul: mostly GPSIMD tt; every Nth on DVE as stt (DVE's
            # plain tt runs ~2x slower than stt on this HW). Separate
            # scratch tags per engine -- a shared tag's slot rotation would
            # serialize the fast engine behind the slow one.
            if i % MUL_DVE_EVERY == 0:
                t_scr = spool.tile([128, M], f32, tag="t_scr_dve",
                                   name="t_scr_dve", bufs=6)
                last_dve2 = _chain(last_dve2, nc.vector.scalar_tensor_tensor(
                    t_scr[:], p1_scr[:], 1.0, sl, AluOp.mult, AluOp.mult))
            else:
                t_scr = spool.tile([128, M], f32, tag="t_scr_gp",
                                   name="t_scr_gp", bufs=12)
                last_gp = _chain(last_gp, nc.gpsimd.tensor_mul(
                    t_scr[:], sl, p1_scr[:]))
            tiles.append((i, sl, t_scr))
        for i, sl, t_scr in tiles:
            last_dve2 = _chain(last_dve2, nc.vector.scalar_tensor_tensor(
                sl, a_b[:], w_tm[:, i:i + 1],
                t_scr[:], AluOp.mult, AluOp.add))
        nc.sync.dma_start(mem_out_v[b],
                          rb.rearrange("p (j m) -> p j m", m=M))


def _fallback_outputs(nc, res, mem_out_v, w_out_v, pw_pm, NB):
    """Truncated-pipeline stand-in writes so every output is produced."""
    for b in range(NB):
        nc.sync.dma_start(mem_out_v[b],
                          res[b].rearrange("p (j m) -> p j m", m=M))
    nc.sync.dma_start(w_out_v, pw_pm[:])


_NC_CACHE = {}


def _get_nc(R, stage=4):
    key = (R, stage)
    if key not in _NC_CACHE:
        _NC_CACHE[key] = build_nc(R, stage)
    return _NC_CACHE[key]


def _sel_matrix(c):
    sel = np.zeros((8, 4), dtype=np.float32)
    sel[(c - 1) % N_CORES, 0] = 1.0   # left neighbor
    sel[(c + 1) % N_CORES, 1] = 1.0   # right neighbor
    sel[:, 2] = 1.0                   # sums
    return sel


def _mats():
    """[identity | shift-down | shift-up] packed as one (128, 384) input."""
    m = np.zeros((128, 384), dtype=np.float32)
    m[:, 0:128] = np.eye(128, dtype=np.float32)
    m[:, 128:256] = np.eye(128, k=1, dtype=np.float32)   # shd[k,m]=1, k==m-1
    m[:, 256:384] = np.eye(128, k=-1, dtype=np.float32)  # shu[k,m]=1, k==m+1
    return m


def _in_maps(memory, k, g, s, gamma, prev_w, e, a):
    N = memory.shape[0]
    R = N // N_CORES
    f4 = np.float32
    base = {
        "k_in": np.ascontiguousarray(k, dtype=f4),
        "e_in": np.ascontiguousarray(e, dtype=f4),
        "a_in": np.ascontiguousarray(a, dtype=f4),
        "g_in": np.ascontiguousarray(g, dtype=f4),
        "s_in": np.ascontiguousarray(s, dtype=f4),
        "gam_in": np.ascontiguousarray(gamma, dtype=f4),
    }
    maps = []
    for c in range(N_CORES):
        m = dict(base)
        m["mem_in"] = np.ascontiguousarray(memory[c * R:(c + 1) * R],
                                           dtype=f4)
        m["pw_in"] = np.ascontiguousarray(prev_w[c * R:(c + 1) * R],
                                          dtype=f4)
        m["sel_in"] = _sel_matrix(c)
        m["mats_in"] = _mats()
        maps.append(m)
    return R, maps


def run(memory, k, g, s, gamma, prev_w, e, a, stage=4, trace=False):
    R, in_maps = _in_maps(memory, k, g, s, gamma, prev_w, e, a)
    nc = _get_nc(R, stage)
    out = bass_utils.run_bass_kernel_spmd(nc, in_maps,
                                          core_ids=list(range(N_CORES)),
                                          trace=trace)
    new_memory = np.concatenate([out.results[c]["mem_out"]
                                 for c in range(N_CORES)], axis=0)
    w = np.concatenate([out.results[c]["w_out"]
                        for c in range(N_CORES)], axis=0)
    return new_memory, w


def kernel(memory, k, beta, g, s, gamma, prev_w, e, a):
    return run(np.asarray(memory), np.asarray(k), np.asarray(g),
               np.asarray(s), np.asarray(gamma), np.asarray(prev_w),
               np.asarray(e), np.asarray(a))
